# revision 15
# baseline (speedup 1.0000x reference)
"""Causal multi-head attention kernel for Trainium2 (Bass/Tile), 8 NeuronCores.

Problem: x[B=4,C=2048,D=1024], Q/K[dq=64,D,H=16], V[dv=64,D,H], W[D,dv,H].
Sharding: 8 shards = (batch b, half of heads). Each core computes the partial
output sum over its 8 heads for its batch; host adds the two half-head
partials per batch. No on-device collectives.

All matmul operands are bf16 (f32r streams ~1.6x slower per column on the PE
array). The softmax denominator rides as a 65th lhsT column of ones in the
P@V matmul, so no separate denominator matmuls are needed. Normalization is
reciprocal on the den row + a partition-broadcast DMA + two DVE multiplies.

Per-core layouts (host-prepared so every DMA is contiguous, all bf16):
  xT  [128, 8*C]    xT[p, j*C + c]   = x[b, c, j*128+p]
  QW  [128, 4*1024] per head-pair pp, 8 d-chunks of [128,128] lhsT tiles,
                    cols m<64 -> head 2pp, m>=64 -> head 2pp+1 (scale folded)
  KW  same layout, unscaled
  VW  [128, 8*512]  VW[p, j*512 + (h*64+vi)] = V[vi, j*128+p, hg+h] * sv
  WC  [128, 4*1024] WC[p, pp*1024 + d] = W[d, p%64, hg+2*pp+p//64] * sw
  maskT [128,128]   maskT[p, s] = 1.0 if s >= p else 0.0
Output z [C, D] bf16 partial (sum over the core's 8 heads).
"""

import math
import numpy as np

# ---------------------------------------------------------------- constants
B, C, D = 4, 2048, 1024
DQ = DV = 64
H = 16
NCORES = 8
P = 128
CQ = 512                      # query block (free dim of S^T tiles)
NJ = D // P                   # 8 d-chunks
NPAIR = 4                     # head pairs per core
U = 8 * 65                    # v8 columns per key chunk (8 heads x (64 v + 1 ones))

_nc_cache = {}


_MAXW = 1  # this walrus build rejects instructions with >1 sem wait


def _patch_tile_tail_drain(tile_mod, bass_rust, ScopedClock):
    """Work around a walrus limit on sync waits per instruction: keep at most
    _MAXW waits on any instruction; hoist the overflow onto same-engine nops
    emitted just before it (same-engine streams are sequential, so blocking at
    an earlier nop is equivalent)."""
    if getattr(tile_mod.TileContext, "_drain_patched", False):
        return

    orig_add = tile_mod.TileContext._add_instruction

    def _add_instruction(self, inst):
        si = getattr(inst, "sync_info", None)
        if si is not None and si.on_wait and len(si.on_wait) > _MAXW:
            waits = list(si.on_wait)
            si.on_wait = waits[:_MAXW]
            overflow = waits[_MAXW:]
            for i in range(0, len(overflow), _MAXW):
                nop = bass_rust.InstNoOp(
                    name=self.nc.get_next_instruction_name(), ins=[], outs=[]
                )
                nop.engine = inst.engine
                nop.sync_info = bass_rust.SyncInfo(
                    on_wait=overflow[i : i + _MAXW], on_update=[]
                )
                orig_add(self, nop)
        orig_add(self, inst)

    def _drain_and_barrier(self, tick_clock, wait_clock):
        nc = self.nc
        drain_inst = nc.sync.drain()
        wait_clock.add_sem_waits(
            drain_inst.ins, ScopedClock({None: tick_clock.global_clock})
        )
        si = drain_inst.ins.sync_info
        waits = list(si.on_wait) if si is not None and si.on_wait else []
        if len(waits) > 1:
            si.on_wait = waits[:1]
            for w in waits[1:]:
                extra = nc.sync.drain()
                esi = extra.ins.sync_info
                if esi is None:
                    extra.ins.sync_info = bass_rust.SyncInfo(
                        on_wait=[w], on_update=[]
                    )
                else:
                    esi.on_wait = list(esi.on_wait) + [w]
        nc.all_engine_barrier()
        popped = nc._tile_sem_poison_stack.pop()
        assert popped is self._sem_poison
        nc.clear_and_free_semaphores(list(self.sems.allocated().values()))
        nc.all_engine_barrier()

    tile_mod.TileContext._add_instruction = _add_instruction
    tile_mod.TileContext._drain_and_barrier = _drain_and_barrier
    tile_mod.TileContext._drain_patched = True


def build_nc(c_total=C):
    """Build the single-core Bass program (SPMD across 8 cores)."""
    import bass_rust
    import concourse.bass as bass
    import concourse.mybir as mybir
    import concourse.tile as tile
    from concourse.vector_clock import ScopedClock

    _patch_tile_tail_drain(tile, bass_rust, ScopedClock)

    f32 = mybir.dt.float32
    bf16 = mybir.dt.bfloat16
    Alu = mybir.AluOpType
    Act = mybir.ActivationFunctionType

    NCQ = c_total // CQ           # query blocks
    NCK = c_total // P            # key chunks

    nc = bass.Bass()
    xT_d = nc.declare_dram_parameter("xT", [P, NJ * c_total], bf16, isOutput=False)
    QW_d = nc.declare_dram_parameter("QW", [P, NPAIR * 1024], bf16, isOutput=False)
    KW_d = nc.declare_dram_parameter("KW", [P, NPAIR * 1024], bf16, isOutput=False)
    VW_d = nc.declare_dram_parameter("VW", [P, NJ * 512], bf16, isOutput=False)
    WC_d = nc.declare_dram_parameter("WC", [P, NPAIR * 1024], bf16, isOutput=False)
    mask_d = nc.declare_dram_parameter("maskT", [P, P], bf16, isOutput=False)
    z_d = nc.declare_dram_parameter("z", [c_total, D], bf16, isOutput=True)

    from contextlib import ExitStack

    with ExitStack() as stack:
        tc = stack.enter_context(tile.TileContext(nc))
        ep = stack.enter_context
        sb = ep(tc.tile_pool(name="sb_singles", bufs=1))
        pool_pt = ep(tc.tile_pool(name="sb_pt", bufs=3))
        pool_rcp = ep(tc.tile_pool(name="sb_rcp", bufs=2))
        pool_dt = ep(tc.tile_pool(name="sb_dt", bufs=2))
        pool_bc = ep(tc.tile_pool(name="sb_bc", bufs=2))
        pool_zo = ep(tc.tile_pool(name="sb_zo", bufs=2))
        ps_s = ep(tc.tile_pool(name="ps_s", bufs=2, space="PSUM"))
        ps_y = ep(tc.tile_pool(name="ps_y", bufs=4, space="PSUM"))

        # ---------------- phase 0: loads + constants
        xt = sb.tile([P, NJ * c_total], bf16, tag="xt")
        for j in range(NJ):
            nc.sync.dma_start(
                out=xt[:, j * c_total : (j + 1) * c_total],
                in_=xT_d[:, j * c_total : (j + 1) * c_total],
            )
        mask = sb.tile([P, P], bf16, tag="mask")
        nc.sync.dma_start(out=mask[:], in_=mask_d[:])
        vw = sb.tile([P, NJ * 512], bf16, tag="vw")
        nc.sync.dma_start(out=vw[:], in_=VW_d[:])
        qw = sb.tile([P, NPAIR * 1024], bf16, tag="qw")
        nc.sync.dma_start(out=qw[:], in_=QW_d[:])
        kw = sb.tile([P, NPAIR * 1024], bf16, tag="kw")
        nc.sync.dma_start(out=kw[:], in_=KW_d[:])
        wc = sb.tile([P, NPAIR * 1024], bf16, tag="wc")
        nc.sync.dma_start(out=wc[:], in_=WC_d[:])

        # ---------------- phase 1: projections (PE), PSUM drained on Scalar
        # v8[cc][c_local, 8 heads x (64 v + ones col)]; ones col feeds the
        # denominator row of the P@V matmul. 64 pad cols so every y-matmul
        # lhsT can be 128 wide (enables fast weight load); rows 65+ of the
        # y PSUM are junk and never read.
        v8 = sb.tile([P, NCK * U + 64], bf16, tag="v8")
        nc.vector.memset(v8[:], 1.0)
        for cc in range(NCK):
            vp = ps_s.tile([P, 1024], f32, tag="s")
            for j in range(NJ):
                nc.tensor.matmul(
                    vp[:, 0:512],
                    lhsT=xt[:, j * c_total + cc * P : j * c_total + (cc + 1) * P],
                    rhs=vw[:, j * 512 : (j + 1) * 512],
                    start=(j == 0),
                    stop=(j == NJ - 1),
                )
            dst = v8[:, cc * U : (cc + 1) * U].rearrange("p (h u) -> p h u", h=8)
            nc.scalar.copy(
                dst[:, :, 0:64],
                vp[:, 0:512].rearrange("p (h u) -> p h u", h=8),
            )

        qt = sb.tile([P, NPAIR * c_total], bf16, tag="qt")
        kt = sb.tile([P, NPAIR * c_total], bf16, tag="kt")
        for pp in range(NPAIR):
            for wt, dst in ((qw, qt), (kw, kt)):
                for sb2 in range(NCQ // 2):     # two 512-col halves per tile
                    pr = ps_s.tile([P, 1024], f32, tag="s")
                    for hf in range(2):
                        c0 = sb2 * 1024 + hf * 512
                        for j in range(NJ):
                            nc.tensor.matmul(
                                pr[:, hf * 512 : (hf + 1) * 512],
                                lhsT=wt[:, pp * 1024 + j * P : pp * 1024 + (j + 1) * P],
                                rhs=xt[:, j * c_total + c0 : j * c_total + c0 + 512],
                                start=(j == 0),
                                stop=(j == NJ - 1),
                            )
                    nc.scalar.copy(
                        dst[:, pp * c_total + sb2 * 1024 : pp * c_total + (sb2 + 1) * 1024],
                        pr[:],
                    )

        # ---------------- phase 2: attention per head-pair, transposed-S flash
        yt = sb.tile([P, NPAIR * c_total], bf16, tag="yt")
        for pp in range(NPAIR):
            qtp = qt[:, pp * c_total : (pp + 1) * c_total]
            ktp = kt[:, pp * c_total : (pp + 1) * c_total]
            for b in range(NCQ):
                nck = 4 * b + 4          # causal: key chunks for this block
                yA = ps_y.tile([P, CQ], f32, tag="y")
                yB = ps_y.tile([P, CQ], f32, tag="y")
                for ck in range(nck):
                    diag = ck >= 4 * b
                    d0 = (ck - 4 * b) * P if diag else 0
                    s_ps = ps_s.tile([P, 1024], f32, tag="s")
                    # S^T = kT.T @ qT, both heads (row-split PE tiles)
                    nc.tensor.matmul(
                        s_ps[:, d0:512],
                        lhsT=ktp[0:64, ck * P : (ck + 1) * P],
                        rhs=qtp[0:64, b * CQ + d0 : (b + 1) * CQ],
                        start=True, stop=True,
                        tile_position=(0, 0),
                    )
                    nc.tensor.matmul(
                        s_ps[:, 512 + d0 : 1024],
                        lhsT=ktp[64:128, ck * P : (ck + 1) * P],
                        rhs=qtp[64:128, b * CQ + d0 : (b + 1) * CQ],
                        start=True, stop=True,
                        tile_position=(64, 0),
                    )
                    # exp for both heads in one ACT instruction (windowed)
                    pt = pool_pt.tile([P, 1024], bf16, tag="pt")
                    s3 = s_ps.rearrange("p (h q) -> p h q", h=2)[:, :, d0:512]
                    p3 = pt.rearrange("p (h q) -> p h q", h=2)[:, :, d0:512]
                    nc.scalar.activation(p3, s3, Act.Exp)
                    # causal wedge masking on the diagonal chunk
                    if diag:
                        nc.vector.tensor_mul(
                            pt[:, d0 : d0 + P], pt[:, d0 : d0 + P], mask[:]
                        )
                        nc.vector.tensor_mul(
                            pt[:, 512 + d0 : 512 + d0 + P],
                            pt[:, 512 + d0 : 512 + d0 + P],
                            mask[:],
                        )
                    # y^T accumulation; lhsT col 64 is ones -> row 64 = denom
                    uA = ck * U + (2 * pp) * 65
                    uB = ck * U + (2 * pp + 1) * 65
                    nc.tensor.matmul(
                        yA[:, d0:CQ],
                        lhsT=v8[:, uA : uA + 128],
                        rhs=pt[:, d0:512],
                        start=(ck == 0), stop=(ck == nck - 1),
                        skip_group_check=True,
                    )
                    nc.tensor.matmul(
                        yB[:, d0:CQ],
                        lhsT=v8[:, uB : uB + 128],
                        rhs=pt[:, 512 + d0 : 1024],
                        start=(ck == 0), stop=(ck == nck - 1),
                        skip_group_check=True,
                    )
                # normalize: yt[:, block] = y * (1/den), den = row 64 of y
                # DVE's iterative divide costs ~6.5 cyc per free-dim column,
                # independent of partition count — so spread each 512-wide den
                # row over 128 partitions (DMA reshape) before reciprocal,
                # then gather back to a row for the partition-broadcast.
                den = pool_rcp.tile([P, CQ], f32, tag="den")
                nc.vector.tensor_copy(den[0:1, :], yA[64:65, :])
                nc.vector.tensor_copy(den[64:65, :], yB[64:65, :])
                dT = pool_dt.tile([P, 8], f32, tag="dt")
                nc.sync.dma_start(out=dT[:, 0:4], in_=den[0:1, :])
                nc.sync.dma_start(out=dT[:, 4:8], in_=den[64:65, :])
                rT = pool_dt.tile([P, 8], f32, tag="rt")
                nc.vector.reciprocal(rT[:], dT[:])
                rcp = pool_rcp.tile([P, CQ], f32, tag="rcp")
                nc.sync.dma_start(out=rcp[0:1, :], in_=rT[:, 0:4])
                nc.sync.dma_start(out=rcp[64:65, :], in_=rT[:, 4:8])
                bc = pool_bc.tile([P, CQ], f32, tag="bc")
                for hh in range(2):
                    row = rcp[hh * 64 : hh * 64 + 1, :]
                    # replicate one SBUF row into 64 partitions: zero-stride
                    # FREE dim on the source (partition dim must keep stride)
                    src = bass.AP(
                        tensor=row.tensor,
                        offset=row.offset,
                        ap=[list(row.ap[0]), [0, 64]] + [list(a) for a in row.ap[1:]],
                    )
                    nc.sync.dma_start(out=bc[hh * 64 : (hh + 1) * 64, :], in_=src)
                nc.vector.scalar_tensor_tensor(
                    yt[0:64, pp * c_total + b * CQ : pp * c_total + (b + 1) * CQ],
                    in0=yA[0:64, :],
                    scalar=1.0,
                    in1=bc[0:64, :],
                    op0=Alu.mult,
                    op1=Alu.mult,
                )
                nc.vector.scalar_tensor_tensor(
                    yt[64:128, pp * c_total + b * CQ : pp * c_total + (b + 1) * CQ],
                    in0=yB[0:64, :],
                    scalar=1.0,
                    in1=bc[64:128, :],
                    op0=Alu.mult,
                    op1=Alu.mult,
                )

        # ---------------- phase 3: output projection (contract all pairs)
        for cc in range(NCK):
            zp = ps_s.tile([P, 1024], f32, tag="s")
            for dd in range(2):
                for pp in range(NPAIR):
                    nc.tensor.matmul(
                        zp[:, dd * 512 : (dd + 1) * 512],
                        lhsT=yt[:, pp * c_total + cc * P : pp * c_total + (cc + 1) * P],
                        rhs=wc[:, pp * 1024 + dd * 512 : pp * 1024 + (dd + 1) * 512],
                        start=(pp == 0),
                        stop=(pp == NPAIR - 1),
                    )
            zo = pool_zo.tile([P, 1024], bf16, tag="zo")
            nc.scalar.copy(zo[:], zp[:])
            nc.sync.dma_start(
                out=z_d[cc * P : (cc + 1) * P, :],
                in_=zo[:],
            )
    return nc


# ---------------------------------------------------------------- host side

def shard_inputs(x, Q, K, V, W, c_total=C):
    """Build the per-core input maps (8 cores: (batch, head-half))."""
    import ml_dtypes

    bf16 = ml_dtypes.bfloat16
    x = np.ascontiguousarray(x, dtype=np.float32)
    Q = np.asarray(Q, dtype=np.float32)
    K = np.asarray(K, dtype=np.float32)
    V = np.asarray(V, dtype=np.float32)
    W = np.asarray(W, dtype=np.float32)

    scale_qk = (DQ / D) / DQ            # sq^2 / dq, folded into Q
    sv = math.sqrt(DV / D)
    sw = math.sqrt(D / DV) / H

    maskT = (np.arange(P)[None, :] >= np.arange(P)[:, None]).astype(bf16)

    in_maps = []
    for core in range(NCORES):
        b = core // 2
        hg = (core % 2) * 8
        xb = x[b, :c_total]                                   # [C, D]
        xT = np.ascontiguousarray(
            xb.T.reshape(NJ, P, c_total).transpose(1, 0, 2).reshape(P, NJ * c_total)
        ).astype(bf16)
        # QW/KW: per pair, [d, hh, m64] -> [128, pair*8 chunks of 128]
        def pack_qk(M, scale):
            out = np.empty((P, NPAIR * 1024), np.float32)
            for pp in range(NPAIR):
                g = M[:, :, hg + 2 * pp : hg + 2 * pp + 2]    # [64, D, 2]
                arr = g.transpose(1, 2, 0).reshape(NJ, P, 128)
                out[:, pp * 1024 : (pp + 1) * 1024] = (
                    arr.transpose(1, 0, 2).reshape(P, 1024) * scale
                )
            return out.astype(bf16)

        QW = pack_qk(Q, scale_qk)
        KW = pack_qk(K, 1.0)
        Vg = V[:, :, hg : hg + 8]                              # [64, D, 8]
        VW = (
            (Vg.transpose(1, 2, 0).reshape(NJ, P, 512) * sv)
            .transpose(1, 0, 2)
            .reshape(P, NJ * 512)
        ).astype(bf16)
        Wg = W[:, :, hg : hg + 8]                              # [D, 64, 8]
        WC = np.empty((P, NPAIR * 1024), np.float32)
        for pp in range(NPAIR):
            wp = Wg[:, :, 2 * pp : 2 * pp + 2].transpose(2, 1, 0).reshape(P, D)
            WC[:, pp * 1024 : (pp + 1) * 1024] = wp * sw
        in_maps.append(
            {
                "xT": np.ascontiguousarray(xT),
                "QW": np.ascontiguousarray(QW),
                "KW": np.ascontiguousarray(KW),
                "VW": np.ascontiguousarray(VW),
                "WC": np.ascontiguousarray(WC.astype(bf16)),
                "maskT": maskT,
            }
        )
    return in_maps


def kernel(x, Q, K, V, W):
    from concourse.bass_utils import run_bass_kernel_spmd

    if "nc" not in _nc_cache:
        _nc_cache["nc"] = build_nc(C)
    nc = _nc_cache["nc"]
    in_maps = shard_inputs(x, Q, K, V, W)
    res = run_bass_kernel_spmd(nc, in_maps, list(range(NCORES)))
    out = np.zeros((B, C, D), np.float32)
    for core in range(NCORES):
        out[core // 2] += np.asarray(res.results[core]["z"], dtype=np.float32)
    return out
